# revision 9
# baseline (speedup 1.0000x reference)
"""Trainium2 Bass kernel: gated MoE residual block (two 3x3 convs, C=32).

  g  = gate * (gate > 0)                          # [B, C]
  h  = relu((conv3x3(x, w1) + b1) * g)
  h2 = relu((conv3x3(h, w2) + b2) * g)
  out = h2 + x

Sharding: data-parallel over batch. 16 images -> 8 cores x 2 images.

Device algorithm (per core, per image), all-fp8 DoubleRow matmuls:
  - x arrives pre-packed in "mod-4 row-interleaved" SBUF layout at fp8_e4m3
    (scale SX=4): partition 32*s+ci, slot t, col u = x[ci, 4(t-1)+s, u-1]*SX,
    zero halo baked in (slots 0, 65; cols 0, 257).
  - conv as fp8 DoubleRow matmuls (0.5 cyc/row, K_eff=256): for window k
    (out rows 4k+1+q, q = out partition group), the pair dim spans slots
    k+1 (main: dy = s-q) and k+2 (wrap: dy = 4+s'-q), one matmul per dx
    with rhs = src[:, k+1:k+3, dx:dx+W]. 3 matmuls x 256 cols per 4 rows.
  - scales chosen so every epilogue is scale-free (SW1*SX == SH,
    SW2*SH == SO): conv1 ep = max(ps + SH*b1, 0) -> h fp8; conv2 ep =
    max(ps + SO*b2, 0) -> out fp8. Single tensor_scalar (DVE/Pool) or
    activation (ScalarE) per 2-window PSUM bank; round-robined across all
    three engines so none backpressures the PE.
  - g >= 0 folds the gate between the convs into w2's input-channel
    columns (per-image wv2 upload); the output gate and +x residual are
    applied on host in f32.
  - all DMA (x in, out stores) issues from SP, which is otherwise idle;
    the cost model charges transfer time to the issuing engine.
"""

import numpy as np
import ml_dtypes

import concourse.bass as bass
import concourse.tile as tile
from concourse import bacc, mybir

B, C, H, W = 16, 32, 256, 256
KW = 3
S = 4            # row interleave factor
A = H // S       # 64 aligned 4-row windows
WP = W + 2       # padded row width
NS = A + 2       # x_il/h_il slots; slot 0 and A+1 zero
NSO = A + 1      # out_stage slots (out row 4(i-1)+2+q at slot i)
IMGS = 2
N_CORES = 8
F32 = mybir.dt.float32
FP8 = mybir.dt.float8e4
E4 = ml_dtypes.float8_e4m3
DR = mybir.MatmulPerfMode.DoubleRow

SX, SW1, SH, SW2 = 4.0, 16.0, 64.0, 2.0
SO = SH * SW2    # 128


def _q8(a):
    return np.asarray(a, dtype=E4)


def _pack_w(w, scale):
    """w [C,C,3,3] OIHW -> [128, 3, 2, 128] fp8 DoubleRow lhsT (main, wrap) per dx."""
    wv = np.zeros((128, KW, 2, 128), np.float32)
    for dx in range(KW):
        for q in range(S):
            for s in range(S):
                if 0 <= s - q <= 2:
                    wv[32 * s:32 * s + 32, dx, 0, 32 * q:32 * q + 32] = \
                        w[:, :, s - q, dx].T * scale
        for sp, q, dy in ((0, 2, 2), (0, 3, 1), (1, 3, 2)):
            wv[32 * sp:32 * sp + 32, dx, 1, 32 * q:32 * q + 32] = \
                w[:, :, dy, dx].T * scale
    return _q8(wv)


def _interleave_x(x):
    """x [n,C,H,W] f32 -> [n,128,NS,WP] fp8: slot t part 32s+ci col u =
    x[ci, 4(t-1)+s, u-1]*SX, zero halo."""
    n = x.shape[0]
    xq = _q8(x * SX)
    out = np.zeros((n, 128, NS, WP), E4)
    v = xq.reshape(n, C, A, S, W).transpose(0, 3, 1, 2, 4).reshape(n, 128, A, W)
    out[:, :, 1:A + 1, 1:1 + W] = v
    return np.ascontiguousarray(out)


def _deinterleave(dev):
    """dev [n,128,NSO,W] fp8 (row 4(i-1)+2+q at slot i part 32q+co) -> [n,C,H,W] f32."""
    dev = np.asarray(dev).astype(np.float32)
    n = dev.shape[0]
    v = dev.reshape(n, S, C, NSO, W).transpose(0, 2, 3, 1, 4) \
           .reshape(n, C, S * NSO, W)
    return np.ascontiguousarray(v[:, :, 2:2 + H, :])


def _build_core_graph():
    nc = bacc.Bacc(None, target_bir_lowering=False, debug=False)

    xil_ext = nc.declare_dram_parameter("xil", [IMGS, 128, NS, WP], FP8, isOutput=False)
    wv1_ext = nc.declare_dram_parameter("wv1", [128, KW, 2, 128], FP8, isOutput=False)
    wv2_ext = nc.declare_dram_parameter("wv2", [128, IMGS, KW, 2, 128], FP8, isOutput=False)
    b1s_ext = nc.declare_dram_parameter("b1s", [128, 1], F32, isOutput=False)
    b2s_ext = nc.declare_dram_parameter("b2s", [128, 1], F32, isOutput=False)
    out_ext = nc.declare_dram_parameter("out", [IMGS, 128, NSO, W], FP8, isOutput=True)

    RELU = mybir.ActivationFunctionType.Relu
    ADD, MAX = mybir.AluOpType.add, mybir.AluOpType.max

    with tile.TileContext(nc) as tc:
        with (
            tc.tile_pool(name="const", bufs=1) as cpool,
            tc.tile_pool(name="xb", bufs=2) as xpool,
            tc.tile_pool(name="hb", bufs=2) as hpool,
            tc.tile_pool(name="os", bufs=2) as ospool,
            tc.tile_pool(name="ps1", bufs=4, space=bass.MemorySpace.PSUM) as ps1pool,
            tc.tile_pool(name="ps2", bufs=4, space=bass.MemorySpace.PSUM) as ps2pool,
        ):
            wv1_t = cpool.tile([128, KW, 2, 128], FP8)
            wv2_t = cpool.tile([128, IMGS, KW, 2, 128], FP8)
            b1s_t = cpool.tile([128, 1], F32)
            b2s_t = cpool.tile([128, 1], F32)

            # PE clock-ramp warm-up on zeroed scratch (results unread)
            warm = cpool.tile([128, 2, 256], FP8, tag="warm")
            nc.vector.memset(warm[:], 0.0)
            wps = ps2pool.tile([128, 2, 256], F32, tag="ps")
            for _ in range(8):
                nc.tensor.matmul(wps[:, 0, :], warm[:, :, 0:128], warm[:],
                                 start=True, stop=True, perf_mode=DR,
                                 skip_group_check=True)
            # eat ScalarE's one-time activation-table load off critical path
            nc.scalar.activation(warm[0:32, 0, 0:1], wps[0:32, 0, 0:1], RELU)

            # weights + biases from Pool at t0 (transfer blocks the issuer)
            nc.gpsimd.dma_start(out=wv1_t[:], in_=wv1_ext[:])
            nc.gpsimd.dma_start(out=b1s_t[:], in_=b1s_ext[:])
            nc.gpsimd.dma_start(out=b2s_t[:], in_=b2s_ext[:])
            nc.gpsimd.dma_start(out=wv2_t[:], in_=wv2_ext[:])

            # x for both images, in first-need order, all on SP
            x_ils = [xpool.tile([128, NS, WP], FP8, name=f"x_il{i}", tag="x")
                     for i in range(IMGS)]
            for img in range(IMGS):
                for c0, c1 in ((0, 8), (8, 20), (20, 40), (40, NS)):
                    nc.sync.dma_start(out=x_ils[img][:, c0:c1, :],
                                      in_=xil_ext[img, :, c0:c1, :])

            ENGS = (nc.scalar, nc.vector, nc.gpsimd)

            def ep(eng, dst, src, bias):
                if eng is nc.scalar:
                    eng.activation(dst, src, RELU, bias=bias)
                else:
                    eng.tensor_scalar(dst, src, bias, 0.0, ADD, MAX)

            for img in range(IMGS):
                x_il = x_ils[img]
                h_il = hpool.tile([128, NS, WP], FP8)
                o_st = ospool.tile([128, NSO, W], FP8)

                # h halo: rows -3..-1 (slot 0 q<3), row 256 (slot A q=3),
                # rows 257.. (slot A+1), dx halo cols (Pool: memset eff 1.0)
                nc.gpsimd.memset(h_il[0:96, 0, :], 0.0)
                nc.gpsimd.memset(h_il[96:128, A, :], 0.0)
                nc.gpsimd.memset(h_il[:, A + 1, :], 0.0)
                nc.gpsimd.memset(h_il[:, :, 0], 0.0)
                nc.gpsimd.memset(h_il[:, :, WP - 1], 0.0)

                def mmgroup(ps, j, src, wv, k):
                    for dx in range(KW):
                        nc.tensor.matmul(ps[:, j, :], wv[:, dx],
                                         src[:, k + 1:k + 3, dx:dx + W],
                                         start=(dx == 0), stop=(dx == KW - 1),
                                         perf_mode=DR, skip_group_check=True)

                # ---- conv1: x_il -> h_il (+1 row phase) ----
                ps = ps1pool.tile([128, 2, 256], F32, tag="ps")
                mmgroup(ps, 0, x_il, wv1_t, -1)   # row 0 (q=3 only)
                ep(nc.scalar, h_il[96:128, 0, 1:1 + W], ps[96:128, 0, :],
                   b1s_t[96:128])
                for p in range(32):
                    ps = ps1pool.tile([128, 2, 256], F32, tag="ps")
                    mmgroup(ps, 0, x_il, wv1_t, 2 * p)
                    mmgroup(ps, 1, x_il, wv1_t, 2 * p + 1)
                    t0 = 2 * p + 1
                    eng = ENGS[p % 3]
                    if p < 31:
                        ep(eng, h_il[:, t0:t0 + 2, 1:1 + W], ps[:, 0:2, :],
                           b1s_t[:])
                    else:
                        # window 63: q=3 would be row 256 -> keep halo zero
                        ep(eng, h_il[:, t0:t0 + 1, 1:1 + W], ps[:, 0:1, :],
                           b1s_t[:])
                        ep(eng, h_il[0:96, t0 + 1, 1:1 + W], ps[0:96, 1, :],
                           b1s_t[0:96])

                # ---- conv2: h_il -> out_stage (edge garbage rows dropped on host) ----
                wv2i = wv2_t[:, img]
                ps = ps2pool.tile([128, 2, 256], F32, tag="ps")
                mmgroup(ps, 0, h_il, wv2i, -1)    # rows 0,1 (q=2,3)
                ep(nc.gpsimd, o_st[:, 0:1, :], ps[:, 0:1, :], b2s_t[:])
                for p in range(32):
                    ps = ps2pool.tile([128, 2, 256], F32, tag="ps")
                    mmgroup(ps, 0, h_il, wv2i, 2 * p)
                    mmgroup(ps, 1, h_il, wv2i, 2 * p + 1)
                    t0 = 2 * p + 1
                    # last three pairs on three different engines so the
                    # drain epilogues run in parallel
                    eng = {29: nc.scalar, 30: nc.vector, 31: nc.gpsimd}.get(
                        p, ENGS[(p + 1) % 3])
                    if p < 31:
                        ep(eng, o_st[:, t0:t0 + 2, :], ps[:, 0:2, :], b2s_t[:])
                    else:
                        # split the final pair so the last store is tiny
                        ep(eng, o_st[:, t0:t0 + 1, :], ps[:, 0:1, :], b2s_t[:])
                        ep(eng, o_st[:, t0 + 1:t0 + 2, :], ps[:, 1:2, :],
                           b2s_t[:])
                    # chunked stores from SP, spread so the queue is never
                    # backed up at the end; tiny final chunks
                    store = {5: (0, 13), 11: (13, 25), 17: (25, 37),
                             23: (37, 49), 27: (49, 57), 29: (57, 61),
                             30: (61, 63), 31: (63, 65)}.get(p)
                    if store is not None:
                        lo, hi = store
                        nc.sync.dma_start(out=out_ext[img, :, lo:hi, :],
                                          in_=o_st[:, lo:hi, :])

    nc.compile()
    return nc


def _host_prep(x, gate_values, w1, b1, w2, b2):
    x = np.ascontiguousarray(np.asarray(x, dtype=np.float32))
    gate_values = np.asarray(gate_values, dtype=np.float32)
    w1 = np.asarray(w1, dtype=np.float32)
    b1 = np.asarray(b1, dtype=np.float32)
    w2 = np.asarray(w2, dtype=np.float32)
    b2 = np.asarray(b2, dtype=np.float32)

    g = gate_values * (gate_values > 0)                      # [B, C]
    wv1 = _pack_w(w1, SW1)
    b1s = np.ascontiguousarray(np.tile((b1 * SH)[:, None], (S, 1)).astype(np.float32))
    b2s = np.ascontiguousarray(np.tile((b2 * SO)[:, None], (S, 1)).astype(np.float32))

    in_maps = []
    for core in range(N_CORES):
        sl = slice(core * IMGS, (core + 1) * IMGS)
        # gate between the convs folds into w2's input-channel columns
        wv2 = np.stack([_pack_w(w2 * g[core * IMGS + i][None, :, None, None], SW2)
                        for i in range(IMGS)], axis=1)
        in_maps.append({
            "xil": _interleave_x(x[sl]),
            "wv1": wv1, "wv2": wv2,
            "b1s": b1s, "b2s": b2s,
        })
    return in_maps


_NC_CACHE = None


def _get_graph():
    global _NC_CACHE
    if _NC_CACHE is None:
        _NC_CACHE = _build_core_graph()
    return _NC_CACHE


def kernel(x, gate_values, w1, b1, w2, b2, _trace=False, **_ignored):
    from concourse.bass_utils import run_bass_kernel_spmd

    nc = _get_graph()
    in_maps = _host_prep(x, gate_values, w1, b1, w2, b2)
    res = run_bass_kernel_spmd(
        nc, in_maps, core_ids=list(range(N_CORES)), trace=_trace)
    outs = [_deinterleave(res.results[i]["out"]) for i in range(N_CORES)]
    full = np.concatenate(outs, axis=0)
    g = (np.asarray(gate_values, np.float32) *
         (np.asarray(gate_values, np.float32) > 0))
    full = full * (g[:, :, None, None] / SO) + np.asarray(x, np.float32)
    if _trace:
        return full, res
    return full


# revision 10
# speedup vs baseline: 1.0115x; 1.0115x over previous
"""Trainium2 Bass kernel: gated MoE residual block (two 3x3 convs, C=32).

  g  = gate * (gate > 0)                          # [B, C]
  h  = relu((conv3x3(x, w1) + b1) * g)
  h2 = relu((conv3x3(h, w2) + b2) * g)
  out = h2 + x

Sharding: data-parallel over batch. 16 images -> 8 cores x 2 images.

Device algorithm (per core, per image), all-fp8 DoubleRow matmuls:
  - x arrives pre-packed in "mod-4 row-interleaved" SBUF layout at fp8_e4m3
    (scale SX=4): partition 32*s+ci, slot t, col u = x[ci, 4(t-1)+s, u-1]*SX,
    zero halo baked in (slots 0, 65; cols 0, 257).
  - conv as fp8 DoubleRow matmuls (0.5 cyc/row, K_eff=256): for window k
    (out rows 4k+1+q, q = out partition group), the pair dim spans slots
    k+1 (main: dy = s-q) and k+2 (wrap: dy = 4+s'-q), one matmul per dx
    with rhs = src[:, k+1:k+3, dx:dx+W]. 3 matmuls x 256 cols per 4 rows.
  - scales chosen so every epilogue is scale-free (SW1*SX == SH,
    SW2*SH == SO): conv1 ep = max(ps + SH*b1, 0) -> h fp8; conv2 ep =
    max(ps + SO*b2, 0) -> out fp8. Single tensor_scalar (DVE/Pool) or
    activation (ScalarE) per 2-window PSUM bank; round-robined across all
    three engines so none backpressures the PE.
  - g >= 0 folds the gate between the convs into w2's input-channel
    columns (per-image wv2 upload); the output gate and +x residual are
    applied on host in f32.
  - all DMA (x in, out stores) issues from SP, which is otherwise idle;
    the cost model charges transfer time to the issuing engine.
"""

import numpy as np
import ml_dtypes

import concourse.bass as bass
import concourse.tile as tile
from concourse import bacc, mybir

B, C, H, W = 16, 32, 256, 256
KW = 3
S = 4            # row interleave factor
A = H // S       # 64 aligned 4-row windows
WP = W + 2       # padded row width
NS = A + 2       # x_il/h_il slots; slot 0 and A+1 zero
NSO = A + 1      # out_stage slots (out row 4(i-1)+2+q at slot i)
IMGS = 2
N_CORES = 8
F32 = mybir.dt.float32
FP8 = mybir.dt.float8e4
E4 = ml_dtypes.float8_e4m3
DR = mybir.MatmulPerfMode.DoubleRow

SX, SW1, SH, SW2 = 4.0, 16.0, 64.0, 2.0
SO = SH * SW2    # 128


def _q8(a):
    return np.asarray(a, dtype=E4)


def _pack_w(w, scale):
    """w [C,C,3,3] OIHW -> [128, 3, 2, 128] fp8 DoubleRow lhsT (main, wrap) per dx."""
    wv = np.zeros((128, KW, 2, 128), np.float32)
    for dx in range(KW):
        for q in range(S):
            for s in range(S):
                if 0 <= s - q <= 2:
                    wv[32 * s:32 * s + 32, dx, 0, 32 * q:32 * q + 32] = \
                        w[:, :, s - q, dx].T * scale
        for sp, q, dy in ((0, 2, 2), (0, 3, 1), (1, 3, 2)):
            wv[32 * sp:32 * sp + 32, dx, 1, 32 * q:32 * q + 32] = \
                w[:, :, dy, dx].T * scale
    return _q8(wv)


def _interleave_x(x):
    """x [n,C,H,W] f32 -> [n,128,NS,WP] fp8: slot t part 32s+ci col u =
    x[ci, 4(t-1)+s, u-1]*SX, zero halo."""
    n = x.shape[0]
    xq = _q8(x * SX)
    out = np.zeros((n, 128, NS, WP), E4)
    v = xq.reshape(n, C, A, S, W).transpose(0, 3, 1, 2, 4).reshape(n, 128, A, W)
    out[:, :, 1:A + 1, 1:1 + W] = v
    return np.ascontiguousarray(out)


def _deinterleave(dev):
    """dev [n,128,NSO,W] fp8 (row 4(i-1)+2+q at slot i part 32q+co) -> [n,C,H,W] f32."""
    dev = np.asarray(dev).astype(np.float32)
    n = dev.shape[0]
    v = dev.reshape(n, S, C, NSO, W).transpose(0, 2, 3, 1, 4) \
           .reshape(n, C, S * NSO, W)
    return np.ascontiguousarray(v[:, :, 2:2 + H, :])


def _build_core_graph():
    nc = bacc.Bacc(None, target_bir_lowering=False, debug=False)

    xil_ext = nc.declare_dram_parameter("xil", [IMGS, 128, NS, WP], FP8, isOutput=False)
    wv1_ext = nc.declare_dram_parameter("wv1", [128, KW, 2, 128], FP8, isOutput=False)
    wv2_ext = nc.declare_dram_parameter("wv2", [128, IMGS, KW, 2, 128], FP8, isOutput=False)
    b1s_ext = nc.declare_dram_parameter("b1s", [128, 1], F32, isOutput=False)
    b2s_ext = nc.declare_dram_parameter("b2s", [128, 1], F32, isOutput=False)
    out_ext = nc.declare_dram_parameter("out", [IMGS, 128, NSO, W], FP8, isOutput=True)

    RELU = mybir.ActivationFunctionType.Relu
    ADD, MAX = mybir.AluOpType.add, mybir.AluOpType.max

    with tile.TileContext(nc) as tc:
        with (
            tc.tile_pool(name="const", bufs=1) as cpool,
            tc.tile_pool(name="xb", bufs=2) as xpool,
            tc.tile_pool(name="hb", bufs=2) as hpool,
            tc.tile_pool(name="os", bufs=2) as ospool,
            tc.tile_pool(name="ps1", bufs=4, space=bass.MemorySpace.PSUM) as ps1pool,
            tc.tile_pool(name="ps2", bufs=4, space=bass.MemorySpace.PSUM) as ps2pool,
        ):
            wv1_t = cpool.tile([128, KW, 2, 128], FP8)
            wv2_t = cpool.tile([128, IMGS, KW, 2, 128], FP8)
            b1s_t = cpool.tile([128, 1], F32)
            b2s_t = cpool.tile([128, 1], F32)

            # PE clock-ramp warm-up on zeroed scratch (results unread)
            warm = cpool.tile([128, 2, 256], FP8, tag="warm")
            nc.vector.memset(warm[:], 0.0)
            wps = ps2pool.tile([128, 2, 256], F32, tag="ps")
            for _ in range(8):
                nc.tensor.matmul(wps[:, 0, :], warm[:, :, 0:128], warm[:],
                                 start=True, stop=True, perf_mode=DR,
                                 skip_group_check=True)
            # eat ScalarE's one-time activation-table load off critical path
            nc.scalar.activation(warm[0:32, 0, 0:1], wps[0:32, 0, 0:1], RELU)

            # weights + biases from Pool at t0 (transfer blocks the issuer)
            nc.gpsimd.dma_start(out=wv1_t[:], in_=wv1_ext[:])
            nc.gpsimd.dma_start(out=b1s_t[:], in_=b1s_ext[:])
            nc.gpsimd.dma_start(out=b2s_t[:], in_=b2s_ext[:])
            nc.gpsimd.dma_start(out=wv2_t[:], in_=wv2_ext[:])

            # x for both images, in first-need order, all on SP
            x_ils = [xpool.tile([128, NS, WP], FP8, name=f"x_il{i}", tag="x")
                     for i in range(IMGS)]
            for img in range(IMGS):
                for c0, c1 in ((0, 8), (8, 20), (20, 40), (40, NS)):
                    nc.sync.dma_start(out=x_ils[img][:, c0:c1, :],
                                      in_=xil_ext[img, :, c0:c1, :])

            ENGS = (nc.scalar, nc.vector, nc.gpsimd)

            def ep(eng, dst, src, bias):
                if eng is nc.scalar:
                    eng.activation(dst, src, RELU, bias=bias)
                else:
                    eng.tensor_scalar(dst, src, bias, 0.0, ADD, MAX)

            for img in range(IMGS):
                x_il = x_ils[img]
                h_il = hpool.tile([128, NS, WP], FP8)
                o_st = ospool.tile([128, NSO, W], FP8)

                # h halo: rows -3..-1 (slot 0 q<3), row 256 (slot A q=3),
                # rows 257.. (slot A+1), dx halo cols (Pool: memset eff 1.0)
                nc.gpsimd.memset(h_il[0:96, 0, :], 0.0)
                nc.gpsimd.memset(h_il[96:128, A, :], 0.0)
                nc.gpsimd.memset(h_il[:, A + 1, :], 0.0)
                nc.gpsimd.memset(h_il[:, :, 0], 0.0)
                nc.gpsimd.memset(h_il[:, :, WP - 1], 0.0)

                def mmgroup(ps, j, src, wv, k):
                    for dx in range(KW):
                        nc.tensor.matmul(ps[:, j, :], wv[:, dx],
                                         src[:, k + 1:k + 3, dx:dx + W],
                                         start=(dx == 0), stop=(dx == KW - 1),
                                         perf_mode=DR, skip_group_check=True)

                # ---- conv1: x_il -> h_il (+1 row phase) ----
                ps = ps1pool.tile([128, 2, 256], F32, tag="ps")
                mmgroup(ps, 0, x_il, wv1_t, -1)   # row 0 (q=3 only)
                ep(nc.scalar, h_il[96:128, 0, 1:1 + W], ps[96:128, 0, :],
                   b1s_t[96:128])
                for p in range(32):
                    ps = ps1pool.tile([128, 2, 256], F32, tag="ps")
                    mmgroup(ps, 0, x_il, wv1_t, 2 * p)
                    mmgroup(ps, 1, x_il, wv1_t, 2 * p + 1)
                    t0 = 2 * p + 1
                    eng = ENGS[p % 3]
                    if p < 31:
                        ep(eng, h_il[:, t0:t0 + 2, 1:1 + W], ps[:, 0:2, :],
                           b1s_t[:])
                    else:
                        # window 63: q=3 would be row 256 -> keep halo zero
                        ep(eng, h_il[:, t0:t0 + 1, 1:1 + W], ps[:, 0:1, :],
                           b1s_t[:])
                        ep(eng, h_il[0:96, t0 + 1, 1:1 + W], ps[0:96, 1, :],
                           b1s_t[0:96])

                # ---- conv2: h_il -> out_stage (edge garbage rows dropped on host) ----
                wv2i = wv2_t[:, img]
                ps = ps2pool.tile([128, 2, 256], F32, tag="ps")
                mmgroup(ps, 0, h_il, wv2i, -1)    # rows 0,1 (q=2,3)
                ep(nc.gpsimd, o_st[:, 0:1, :], ps[:, 0:1, :], b2s_t[:])
                for p in range(32):
                    ps = ps2pool.tile([128, 2, 256], F32, tag="ps")
                    mmgroup(ps, 0, h_il, wv2i, 2 * p)
                    mmgroup(ps, 1, h_il, wv2i, 2 * p + 1)
                    t0 = 2 * p + 1
                    # last three pairs on three different engines so the
                    # drain epilogues run in parallel
                    eng = {29: nc.scalar, 30: nc.vector, 31: nc.gpsimd}.get(
                        p, ENGS[(p + 1) % 3])
                    if p < 31:
                        ep(eng, o_st[:, t0:t0 + 2, :], ps[:, 0:2, :], b2s_t[:])
                    else:
                        # split the final pair so the last store is tiny
                        ep(eng, o_st[:, t0:t0 + 1, :], ps[:, 0:1, :], b2s_t[:])
                        ep(eng, o_st[:, t0 + 1:t0 + 2, :], ps[:, 1:2, :],
                           b2s_t[:])
                    # chunked stores from SP, spread so the queue is never
                    # backed up at the end (each SP DMA costs >= ~500ns)
                    store = {5: (0, 13), 11: (13, 25), 17: (25, 37),
                             23: (37, 49), 27: (49, 57), 31: (57, 65)}.get(p)
                    if store is not None:
                        lo, hi = store
                        nc.sync.dma_start(out=out_ext[img, :, lo:hi, :],
                                          in_=o_st[:, lo:hi, :])

    nc.compile()
    return nc


def _host_prep(x, gate_values, w1, b1, w2, b2):
    x = np.ascontiguousarray(np.asarray(x, dtype=np.float32))
    gate_values = np.asarray(gate_values, dtype=np.float32)
    w1 = np.asarray(w1, dtype=np.float32)
    b1 = np.asarray(b1, dtype=np.float32)
    w2 = np.asarray(w2, dtype=np.float32)
    b2 = np.asarray(b2, dtype=np.float32)

    g = gate_values * (gate_values > 0)                      # [B, C]
    wv1 = _pack_w(w1, SW1)
    b1s = np.ascontiguousarray(np.tile((b1 * SH)[:, None], (S, 1)).astype(np.float32))
    b2s = np.ascontiguousarray(np.tile((b2 * SO)[:, None], (S, 1)).astype(np.float32))

    in_maps = []
    for core in range(N_CORES):
        sl = slice(core * IMGS, (core + 1) * IMGS)
        # gate between the convs folds into w2's input-channel columns
        wv2 = np.stack([_pack_w(w2 * g[core * IMGS + i][None, :, None, None], SW2)
                        for i in range(IMGS)], axis=1)
        in_maps.append({
            "xil": _interleave_x(x[sl]),
            "wv1": wv1, "wv2": wv2,
            "b1s": b1s, "b2s": b2s,
        })
    return in_maps


_NC_CACHE = None


def _get_graph():
    global _NC_CACHE
    if _NC_CACHE is None:
        _NC_CACHE = _build_core_graph()
    return _NC_CACHE


def kernel(x, gate_values, w1, b1, w2, b2, _trace=False, **_ignored):
    from concourse.bass_utils import run_bass_kernel_spmd

    nc = _get_graph()
    in_maps = _host_prep(x, gate_values, w1, b1, w2, b2)
    res = run_bass_kernel_spmd(
        nc, in_maps, core_ids=list(range(N_CORES)), trace=_trace)
    outs = [_deinterleave(res.results[i]["out"]) for i in range(N_CORES)]
    full = np.concatenate(outs, axis=0)
    g = (np.asarray(gate_values, np.float32) *
         (np.asarray(gate_values, np.float32) > 0))
    full = full * (g[:, :, None, None] / SO) + np.asarray(x, np.float32)
    if _trace:
        return full, res
    return full
